# revision 12
# baseline (speedup 1.0000x reference)
"""Trainium2 kernel: AdaptiveFeaturePooling (attention-gated segment mean-pool + max-pool).

v5 strategy (segment-sharded, no collectives):
  - 1024 segments -> 128 per core. Host pads each segment to T_SEG*128 nodes and
    ships x in bf16 twice: node-major ("xn", loaded with a 9-row interleave so
    partition p holds rows {seg + T_SEG*p + k} -> 2.25KB DMA runs) and d-major
    ("xt" = transpose, 18KB runs). Same total HBM bytes as one f32 pass.
  - Per chunk (8 segments = 9216 nodes):
      dots: 72 PE matmuls (wstack variant r stationary, stride-T_SEG comb of xt
            moving) accumulating a [72 x 128] PSUM block; ACT sigmoid; PE block
            transpose -> sigmoid weight columns [128 x 72].
      segsum: 72 PE matmuls (w column stationary, xn comb tile moving) into
            per-segment [1 x 128] PSUM rows at 32-aligned bases ({0,32,64},
            12 segments per bank, 4 generations of 32 segments).
      max: VE TT-max tree (bf16 2x mode) per segment over the xt slice.
  - Host: mean = seg_sum / counts, assemble [1024, 256] = [max | mean].
"""

import os
import sys

import numpy as np

for _p in ("/opt/trn_rl_repo", "/root/.axon_site/_ro/trn_rl_repo"):
    if os.path.isdir(_p) and _p not in sys.path:
        sys.path.insert(0, _p)

import ml_dtypes  # noqa: E402

N_CORES = 8
G = 1024
SPC = G // N_CORES        # 128 segments per core
D = 128
SEGS_PER_CHUNK = 8
GEN_CHUNKS = 4            # chunks per psum generation (32 segments)


def _seg_slot(i):
    """Map gen-local segment index (0..31) -> (bank, base, colblk)."""
    bank, j = divmod(i, 12)
    return bank, 32 * (j // 4), j % 4


def _build_graph(T_SEG):
    import concourse.bass as bass
    import concourse.tile as tile
    from concourse import bacc, mybir

    f32 = mybir.dt.float32
    bf16 = mybir.dt.bfloat16

    seg_nodes = T_SEG * 128
    chunk_nodes = SEGS_PER_CHUNK * seg_nodes      # 9216 for T_SEG=9
    n_chunks = SPC // SEGS_PER_CHUNK              # 16
    n_gens = n_chunks // GEN_CHUNKS               # 4
    npad = SPC * seg_nodes
    ncombs = SEGS_PER_CHUNK * T_SEG               # combs (MMs) per chunk = 72

    nc = bacc.Bacc(None, target_bir_lowering=False)
    xn_d = nc.dram_tensor("xn", [npad, D], bf16, kind="ExternalInput")
    xt_d = nc.dram_tensor("xt", [D, npad], bf16, kind="ExternalInput")
    ws_d = nc.dram_tensor("wstack", [D, ncombs, ncombs], bf16, kind="ExternalInput")
    bv_d = nc.dram_tensor("bvec", [ncombs, 1], f32, kind="ExternalInput")
    id_d = nc.dram_tensor("ident", [ncombs, ncombs], bf16, kind="ExternalInput")
    osum_d = nc.dram_tensor("o_sum", [n_gens, 9 * 512], f32, kind="ExternalOutput")
    omax_d = nc.dram_tensor("o_maxT", [D, SPC], f32, kind="ExternalOutput")

    with tile.TileContext(nc) as tc:
        with (
            tc.tile_pool(name="const", bufs=1) as constp,
            tc.tile_pool(name="xn", bufs=3) as xnp,
            tc.tile_pool(name="xt", bufs=3) as xtp,
            tc.tile_pool(name="wsg", bufs=2) as wsgp,
            tc.tile_pool(name="wcol", bufs=3) as wcolp,
            tc.tile_pool(name="scr", bufs=2) as scrp,
            tc.tile_pool(name="stage", bufs=2) as stagep,
            tc.tile_pool(name="acc", bufs=1) as accp,
            tc.tile_pool(name="psd", bufs=2, space=bass.MemorySpace.PSUM) as psdp,
            tc.tile_pool(name="psw", bufs=2, space=bass.MemorySpace.PSUM) as pswp,
            tc.tile_pool(name="pseg", bufs=1, space=bass.MemorySpace.PSUM) as psegp,
        ):
            ws_sb = constp.tile([D, ncombs, ncombs], bf16)
            nc.sync.dma_start(ws_sb[:], ws_d[:])
            bv_sb = constp.tile([ncombs, 1], f32)
            nc.sync.dma_start(bv_sb[:], bv_d[:])
            id_sb = constp.tile([ncombs, ncombs], bf16)
            nc.sync.dma_start(id_sb[:], id_d[:])

            omax_sb = accp.tile([D, SPC], f32)

            for g in range(n_gens):
                pseg = [
                    psegp.tile([128, 512], f32, tag=f"pseg{b}", name=f"pseg{b}_{g}")
                    for b in range(3)
                ]
                for cc in range(GEN_CHUNKS):
                    c = g * GEN_CHUNKS + cc
                    row0 = c * chunk_nodes
                    # node-major load with comb interleave:
                    # partition p <- rows {seg*seg_nodes + T_SEG*p + k}
                    xn = xnp.tile([128, SEGS_PER_CHUNK, T_SEG, D], bf16)
                    nc.sync.dma_start(
                        xn[:],
                        xn_d[row0 : row0 + chunk_nodes, :].rearrange(
                            "(s p q) d -> p s q d", p=128, q=T_SEG
                        ),
                    )
                    # d-major load (node order on the free axis)
                    xt = xtp.tile([D, chunk_nodes], bf16)
                    nc.sync.dma_start(xt[:], xt_d[:, row0 : row0 + chunk_nodes])

                    # dots: comb m = (seg e, k) -> psum row m
                    psd = psdp.tile([ncombs, D], f32)
                    for e in range(SEGS_PER_CHUNK):
                        for k in range(T_SEG):
                            m = e * T_SEG + k
                            nc.tensor.matmul(
                                psd[:],
                                ws_sb[:, m, :],
                                xt[:, e * seg_nodes + k : e * seg_nodes + seg_nodes : T_SEG],
                                start=(m == 0),
                                stop=(m == ncombs - 1),
                            )
                    # sigmoid gate -> weight rows, then transpose to columns
                    wsg = wsgp.tile([ncombs, D], bf16)
                    nc.scalar.activation(
                        wsg[:],
                        psd[:],
                        mybir.ActivationFunctionType.Sigmoid,
                        bias=bv_sb[:],
                        scale=1.0,
                    )
                    psw = pswp.tile([D, ncombs], bf16)
                    nc.tensor.transpose(psw[:], wsg[:], id_sb[:])
                    wcol = wcolp.tile([D, ncombs], bf16)
                    nc.scalar.copy(wcol[:], psw[:])

                    # segment max: TT-max tree (2x mode) over each segment's xt slice
                    for e in range(SEGS_PER_CHUNK):
                        sl = c * SEGS_PER_CHUNK + e
                        S = xt[:, e * seg_nodes : (e + 1) * seg_nodes]
                        rem = seg_nodes
                        cur = S
                        while rem > 128:
                            half = 128 * ((rem // 128) // 2)
                            if half == 0:
                                break
                            tail = rem - 2 * half
                            nxt = scrp.tile([D, half], bf16, tag=f"scr{half}")
                            nc.vector.tensor_tensor(
                                nxt[:],
                                cur[:, 0:half],
                                cur[:, half : 2 * half],
                                op=mybir.AluOpType.max,
                            )
                            if tail > 0:
                                nc.vector.tensor_tensor(
                                    nxt[:, 0:tail],
                                    nxt[:, 0:tail],
                                    cur[:, 2 * half : rem],
                                    op=mybir.AluOpType.max,
                                )
                            cur, rem = nxt, half
                        nc.vector.tensor_reduce(
                            omax_sb[:, sl : sl + 1],
                            cur[:, 0:rem],
                            axis=mybir.AxisListType.X,
                            op=mybir.AluOpType.max,
                        )

                    # weighted segment sums: comb (e, k) -> psum row of segment
                    for e in range(SEGS_PER_CHUNK):
                        i = cc * SEGS_PER_CHUNK + e      # gen-local segment index
                        bank, base, colblk = _seg_slot(i)
                        for k in range(T_SEG):
                            m = e * T_SEG + k
                            nc.tensor.matmul(
                                pseg[bank][base : base + 1, 128 * colblk : 128 * (colblk + 1)],
                                wcol[:, m : m + 1],
                                xn[:, e, k, :],
                                start=(k == 0),
                                stop=(k == T_SEG - 1),
                            )

                # stage this generation's sums out of PSUM (single-partition tile;
                # engine partition bases must stay 32-aligned)
                stage = stagep.tile([1, 9 * 512], f32)
                nused = [3, 3, 2]  # bases used per bank (32 = 12+12+8)
                for bank in range(3):
                    for bi in range(nused[bank]):
                        r = bank * 3 + bi
                        nc.scalar.copy(
                            stage[0:1, 512 * r : 512 * (r + 1)],
                            pseg[bank][32 * bi : 32 * bi + 1, :],
                        )
                nc.sync.dma_start(osum_d[g : g + 1, :], stage[:])

            nc.sync.dma_start(omax_d[:], omax_sb[:])

    return nc


def _pack_inputs(x, batch, attn_w, attn_b, T_SEG, bounds):
    bf16 = ml_dtypes.bfloat16
    seg_nodes = T_SEG * 128
    npad = SPC * seg_nodes
    ncombs = SEGS_PER_CHUNK * T_SEG
    w = np.asarray(attn_w, dtype=np.float32).reshape(D)
    wstack_np = np.zeros((D, ncombs, ncombs), dtype=np.float32)
    for r in range(ncombs):
        wstack_np[:, r, r] = w
    wstack_np = wstack_np.astype(bf16)
    bvec_np = np.full((ncombs, 1), float(np.asarray(attn_b).reshape(-1)[0]), np.float32)
    ident_np = np.eye(ncombs, dtype=np.float32).astype(bf16)

    in_maps = []
    for c in range(N_CORES):
        xpad = np.zeros((npad, D), dtype=np.float32)
        for sl in range(SPC):
            s = c * SPC + sl
            n0, n1 = int(bounds[s]), int(bounds[s + 1])
            if n1 > n0:
                r0 = sl * seg_nodes
                xpad[r0 : r0 + (n1 - n0)] = x[n0:n1]
        xb = xpad.astype(bf16)
        xtb = np.ascontiguousarray(xb.T)
        in_maps.append(
            {"xn": xb, "xt": xtb, "wstack": wstack_np, "bvec": bvec_np,
             "ident": ident_np}
        )
    return in_maps


LAST_EXEC_NS = None
LAST_RESULT = None


def kernel(x, edge_index, batch, attn_w, attn_b):
    global LAST_EXEC_NS, LAST_RESULT
    from concourse.bass_utils import run_bass_kernel_spmd

    x = np.asarray(x, dtype=np.float32)
    batch = np.asarray(batch)
    bounds = np.searchsorted(batch, np.arange(G + 1))
    counts = np.diff(bounds)
    T_SEG = max(1, int(np.ceil(counts.max() / 128.0)))

    nc = _build_graph(T_SEG)
    nc.compile()
    in_maps = _pack_inputs(x, batch, attn_w, attn_b, T_SEG, bounds)
    res = run_bass_kernel_spmd(nc, in_maps, core_ids=list(range(N_CORES)))
    LAST_RESULT = res
    LAST_EXEC_NS = getattr(res, "exec_time_ns", None)

    out = np.empty((G, 2 * D), dtype=np.float32)
    for c in range(N_CORES):
        osum = np.asarray(res.results[c]["o_sum"], dtype=np.float32).reshape(-1, 9, 512)
        maxT = np.asarray(res.results[c]["o_maxT"], dtype=np.float32)  # [D, SPC]
        sums = np.empty((SPC, D), dtype=np.float32)
        for sl in range(SPC):
            g, i = divmod(sl, 32)
            bank, base, colblk = _seg_slot(i)
            sums[sl] = osum[g, bank * 3 + base // 32, 128 * colblk : 128 * (colblk + 1)]
        cc = counts[c * SPC : (c + 1) * SPC].astype(np.float32)
        mean = sums / np.maximum(cc, 1.0)[:, None]
        mx = maxT.T.copy()
        mx[cc == 0] = -np.inf
        out[c * SPC : (c + 1) * SPC, :D] = mx
        out[c * SPC : (c + 1) * SPC, D:] = mean
    return out


# revision 15
# speedup vs baseline: 1.5767x; 1.5767x over previous
"""Trainium2 kernel: AdaptiveFeaturePooling (attention-gated segment mean-pool + max-pool).

v6 strategy (segment-sharded, no collectives):
  - 1024 segments -> 128 per core. Host pads each segment to T_SEG*128 nodes and
    ships x in bf16 twice:
      "xn": node-major; DMA loads with a T_SEG-row interleave so partition p
            holds rows {seg + T_SEG*p + k} (comb layout, 2.25KB DMA runs).
      "xt": d-major, columns permuted into the SAME comb order
            (col = seg*seg_nodes + 128k + p), 18KB DMA runs.
    Same total HBM bytes as one f32 pass over the original input.
  - Per chunk (8 segments = 9216 nodes):
      dots: 2*T_SEG PE matmuls, contiguous N=512 rhs from xt, wstack variant r
            stationary, accumulating a [2*T_SEG x 512] PSUM block; ACT sigmoid;
            4 PE block transposes -> sigmoid weight columns [128 x 8*T_SEG]
            (comb columns, strided copy-out).
      segsum: 2*T_SEG PE matmuls; quad of 4 segments per matmul via a
            [128 x 4] block of weight columns (stride-T_SEG slice), rhs
            xn[:, e0:e0+4, k, :] (N=512); accumulates [4 x 512] PSUM regions
            at 32-aligned bases; diagonal 128-blocks are the real sums.
      max: VE TT-max tree (bf16 2x mode) per segment over the xt slice.
  - Host: mean = seg_sum / counts, assemble [1024, 256] = [max | mean].
"""

import os
import sys

import numpy as np

for _p in ("/opt/trn_rl_repo", "/root/.axon_site/_ro/trn_rl_repo"):
    if os.path.isdir(_p) and _p not in sys.path:
        sys.path.insert(0, _p)

import ml_dtypes  # noqa: E402

N_CORES = 8
G = 1024
SPC = G // N_CORES        # 128 segments per core
D = 128
SEGS_PER_CHUNK = 8
GEN_CHUNKS = 4            # chunks per psum generation (32 segments)


def _build_graph(T_SEG):
    import concourse.bass as bass
    import concourse.tile as tile
    from concourse import bacc, mybir

    f32 = mybir.dt.float32
    bf16 = mybir.dt.bfloat16

    seg_nodes = T_SEG * 128
    chunk_nodes = SEGS_PER_CHUNK * seg_nodes      # 9216 for T_SEG=9
    n_chunks = SPC // SEGS_PER_CHUNK              # 16
    n_gens = n_chunks // GEN_CHUNKS               # 4
    npad = SPC * seg_nodes
    nblk = chunk_nodes // 512                     # dot blocks per chunk = 2*T_SEG
    ncombs = SEGS_PER_CHUNK * T_SEG               # weight columns per chunk = 72

    nc = bacc.Bacc(None, target_bir_lowering=False)
    xn_d = nc.dram_tensor("xn", [npad, D], bf16, kind="ExternalInput")
    xt_d = nc.dram_tensor("xt", [D, npad], bf16, kind="ExternalInput")
    ws_d = nc.dram_tensor("wstack", [D, nblk, nblk], bf16, kind="ExternalInput")
    bv_d = nc.dram_tensor("bvec", [nblk, 1], f32, kind="ExternalInput")
    id_d = nc.dram_tensor("ident", [nblk, nblk], bf16, kind="ExternalInput")
    osum_d = nc.dram_tensor("o_sum", [n_gens, 4, 8 * 512], f32, kind="ExternalOutput")
    omax_d = nc.dram_tensor("o_maxT", [D, SPC], f32, kind="ExternalOutput")

    with tile.TileContext(nc) as tc:
        with (
            tc.tile_pool(name="const", bufs=1) as constp,
            tc.tile_pool(name="xn", bufs=3) as xnp,
            tc.tile_pool(name="xt", bufs=3) as xtp,
            tc.tile_pool(name="wsg", bufs=2) as wsgp,
            tc.tile_pool(name="wcol", bufs=3) as wcolp,
            tc.tile_pool(name="scr", bufs=2) as scrp,
            tc.tile_pool(name="stage", bufs=2) as stagep,
            tc.tile_pool(name="acc", bufs=1) as accp,
            tc.tile_pool(name="psd", bufs=2, space=bass.MemorySpace.PSUM) as psdp,
            tc.tile_pool(name="psw", bufs=2, space=bass.MemorySpace.PSUM) as pswp,
            tc.tile_pool(name="pseg", bufs=1, space=bass.MemorySpace.PSUM) as psegp,
        ):
            ws_sb = constp.tile([D, nblk, nblk], bf16)
            nc.sync.dma_start(ws_sb[:], ws_d[:])
            bv_sb = constp.tile([nblk, 1], f32)
            nc.sync.dma_start(bv_sb[:], bv_d[:])
            id_sb = constp.tile([nblk, nblk], bf16)
            nc.sync.dma_start(id_sb[:], id_d[:])

            omax_sb = accp.tile([D, SPC], f32)

            for g in range(n_gens):
                pseg = [
                    psegp.tile([128, 512], f32, tag=f"pseg{b}", name=f"pseg{b}_{g}")
                    for b in range(3)
                ]
                for cc in range(GEN_CHUNKS):
                    c = g * GEN_CHUNKS + cc
                    row0 = c * chunk_nodes
                    # node-major load, comb interleave
                    xn = xnp.tile([128, SEGS_PER_CHUNK, T_SEG, D], bf16)
                    nc.sync.dma_start(
                        xn[:],
                        xn_d[row0 : row0 + chunk_nodes, :].rearrange(
                            "(s p q) d -> p s q d", p=128, q=T_SEG
                        ),
                    )
                    # d-major load (comb-ordered columns)
                    xt = xtp.tile([D, chunk_nodes], bf16)
                    nc.sync.dma_start(xt[:], xt_d[:, row0 : row0 + chunk_nodes])

                    # dots: block r -> psum row r (contiguous N=512 rhs)
                    psd = psdp.tile([nblk, 512], f32)
                    for r in range(nblk):
                        nc.tensor.matmul(
                            psd[:],
                            ws_sb[:, r, :],
                            xt[:, 512 * r : 512 * (r + 1)],
                            start=(r == 0),
                            stop=(r == nblk - 1),
                        )
                    # sigmoid gate, then transpose into comb weight columns
                    wsg = wsgp.tile([nblk, 512], bf16)
                    nc.scalar.activation(
                        wsg[:],
                        psd[:],
                        mybir.ActivationFunctionType.Sigmoid,
                        bias=bv_sb[:],
                        scale=1.0,
                    )
                    wcol = wcolp.tile([D, ncombs], bf16)
                    for q in range(4):
                        psw = pswp.tile([D, nblk], bf16, tag="psw", name=f"psw_{c}_{q}")
                        nc.tensor.transpose(
                            psw[:], wsg[:, 128 * q : 128 * (q + 1)], id_sb[:]
                        )
                        nc.scalar.copy(wcol[:, q : ncombs : 4], psw[:])

                    # segment max: TT-max tree (2x mode) per segment
                    for e in range(SEGS_PER_CHUNK):
                        sl = c * SEGS_PER_CHUNK + e
                        S = xt[:, e * seg_nodes : (e + 1) * seg_nodes]
                        rem = seg_nodes
                        cur = S
                        while rem > 128:
                            half = 128 * ((rem // 128) // 2)
                            if half == 0:
                                break
                            tail = rem - 2 * half
                            nxt = scrp.tile([D, half], bf16, tag=f"scr{half}",
                                            name=f"scr_{sl}_{half}")
                            nc.vector.tensor_tensor(
                                nxt[:], cur[:, 0:half], cur[:, half : 2 * half],
                                op=mybir.AluOpType.max,
                            )
                            if tail > 0:
                                nc.vector.tensor_tensor(
                                    nxt[:, 0:tail], nxt[:, 0:tail],
                                    cur[:, 2 * half : rem],
                                    op=mybir.AluOpType.max,
                                )
                            cur, rem = nxt, half
                        nc.vector.tensor_reduce(
                            omax_sb[:, sl : sl + 1],
                            cur[:, 0:rem],
                            axis=mybir.AxisListType.X,
                            op=mybir.AluOpType.max,
                        )

                    # segment sums: quad of 4 segments per matmul
                    for qd in range(SEGS_PER_CHUNK // 4):
                        e0 = 4 * qd
                        quad = cc * 2 + qd            # gen-local quad index (0..7)
                        bank, base = quad // 3, 32 * (quad % 3)
                        for k in range(T_SEG):
                            m0 = T_SEG * e0 + k
                            nc.tensor.matmul(
                                pseg[bank][base : base + 4, :],
                                wcol[:, m0 : m0 + 3 * T_SEG + 1 : T_SEG],
                                xn[:, e0 : e0 + 4, k, :],
                                start=(k == 0),
                                stop=(k == T_SEG - 1),
                            )

                # stage this generation's quad blocks out of PSUM (host takes
                # the diagonal 128-blocks); partition bases stay 32-aligned
                stage = stagep.tile([4, 8 * 512], f32)
                for quad in range(8):
                    bank, base = quad // 3, 32 * (quad % 3)
                    nc.scalar.copy(
                        stage[0:4, 512 * quad : 512 * (quad + 1)],
                        pseg[bank][base : base + 4, :],
                    )
                nc.sync.dma_start(osum_d[g, :, :], stage[:])

            nc.sync.dma_start(omax_d[:], omax_sb[:])

    return nc


def _pack_inputs(x, batch, attn_w, attn_b, T_SEG, bounds):
    bf16 = ml_dtypes.bfloat16
    seg_nodes = T_SEG * 128
    npad = SPC * seg_nodes
    nblk = SEGS_PER_CHUNK * seg_nodes // 512
    w = np.asarray(attn_w, dtype=np.float32).reshape(D)
    wstack_np = np.zeros((D, nblk, nblk), dtype=np.float32)
    for r in range(nblk):
        wstack_np[:, r, r] = w
    wstack_np = wstack_np.astype(bf16)
    bvec_np = np.full((nblk, 1), float(np.asarray(attn_b).reshape(-1)[0]), np.float32)
    ident_np = np.eye(nblk, dtype=np.float32).astype(bf16)

    in_maps = []
    for c in range(N_CORES):
        xpad = np.zeros((npad, D), dtype=np.float32)
        for sl in range(SPC):
            s = c * SPC + sl
            n0, n1 = int(bounds[s]), int(bounds[s + 1])
            if n1 > n0:
                r0 = sl * seg_nodes
                xpad[r0 : r0 + (n1 - n0)] = x[n0:n1]
        xb = xpad.astype(bf16)
        # comb-ordered transpose: col(seg, k, p) <- row(seg + T_SEG*p + k)
        # xb [SPC, 128(p), T_SEG(k), D] -> [D, SPC, k, p]
        x4 = xb.reshape(SPC, 128, T_SEG, D)
        xtb = np.ascontiguousarray(np.transpose(x4, (3, 0, 2, 1))).reshape(D, npad)
        in_maps.append(
            {"xn": xb, "xt": xtb, "wstack": wstack_np, "bvec": bvec_np,
             "ident": ident_np}
        )
    return in_maps


LAST_EXEC_NS = None
LAST_RESULT = None


def kernel(x, edge_index, batch, attn_w, attn_b):
    global LAST_EXEC_NS, LAST_RESULT
    from concourse.bass_utils import run_bass_kernel_spmd

    x = np.asarray(x, dtype=np.float32)
    batch = np.asarray(batch)
    bounds = np.searchsorted(batch, np.arange(G + 1))
    counts = np.diff(bounds)
    T_SEG = max(1, int(np.ceil(counts.max() / 128.0)))

    nc = _build_graph(T_SEG)
    nc.compile()
    in_maps = _pack_inputs(x, batch, attn_w, attn_b, T_SEG, bounds)
    res = run_bass_kernel_spmd(nc, in_maps, core_ids=list(range(N_CORES)))
    LAST_RESULT = res
    LAST_EXEC_NS = getattr(res, "exec_time_ns", None)

    out = np.empty((G, 2 * D), dtype=np.float32)
    for c in range(N_CORES):
        osum = np.asarray(res.results[c]["o_sum"], dtype=np.float32)   # [gens, 4, 8*512]
        maxT = np.asarray(res.results[c]["o_maxT"], dtype=np.float32)  # [D, SPC]
        sums = np.empty((SPC, D), dtype=np.float32)
        for sl in range(SPC):
            g, i2 = divmod(sl, 32)
            quad, i = divmod(i2, 4)
            sums[sl] = osum[g, i, 512 * quad + D * i : 512 * quad + D * (i + 1)]
        cc = counts[c * SPC : (c + 1) * SPC].astype(np.float32)
        mean = sums / np.maximum(cc, 1.0)[:, None]
        mx = maxT.T.copy()
        mx[cc == 0] = -np.inf
        out[c * SPC : (c + 1) * SPC, :D] = mx
        out[c * SPC : (c + 1) * SPC, D:] = mean
    return out


# revision 16
# speedup vs baseline: 1.7122x; 1.0860x over previous
"""Trainium2 kernel: AdaptiveFeaturePooling (attention-gated segment mean-pool + max-pool).

v7 strategy (segment-sharded, no collectives):
  - 1024 segments -> 128 per core, but the host REASSIGNS segments to cores so
    every core gets an identical mix: n_hi segments padded to T_HI tiles and
    the rest padded to T_LO tiles (T = ceil(len/128) tiles of 128 nodes).
    One uniform SPMD graph with mixed chunk types; ~6% padding instead of 18%.
  - x ships in bf16 twice:
      "xn": node-major; DMA loads with a T-row interleave so partition p holds
            rows {seg_off + T*p + k} (comb layout, >=2KB DMA runs).
      "xt": d-major, columns permuted into the SAME comb order
            (col = seg_off + 128k + p), ~16KB DMA runs.
    Total HBM bytes ~= one f32 pass over the original input.
  - Per chunk (8 segments):
      dots: 2T PE matmuls, contiguous N=512 rhs from xt, wstack variant r
            stationary, accumulating a [2T x 512] PSUM block; ACT sigmoid;
            4 PE block transposes -> comb weight columns [128 x 8T].
      segsum: 2T PE matmuls; quad of 4 segments per matmul via a [128 x 4]
            stride-T slice of weight columns, rhs xn[:, e0:e0+4, k, :] (N=512);
            accumulates [4 x 512] PSUM regions at 32-aligned bases {0,32,64};
            diagonal 128-blocks are the real sums (host extracts).
      max: VE TT-max tree (bf16 2x mode) per segment over the xt slice.
  - Host: mean = seg_sum / counts, un-permute, assemble [1024, 256] = [max | mean].
"""

import os
import sys

import numpy as np

for _p in ("/opt/trn_rl_repo", "/root/.axon_site/_ro/trn_rl_repo"):
    if os.path.isdir(_p) and _p not in sys.path:
        sys.path.insert(0, _p)

import ml_dtypes  # noqa: E402

N_CORES = 8
G = 1024
SPC = G // N_CORES        # 128 segments per core
D = 128
SEGS_PER_CHUNK = 8
GEN_CHUNKS = 4            # chunks per psum generation (32 segments)


def _plan(counts):
    """Segment -> core assignment with a uniform per-core chunk-type list.

    Returns (perm, chunk_ts) where perm[core, j] = global segment id placed at
    per-core slot j, and chunk_ts = list of T values (tiles/segment), one per
    chunk of 8 slots (same for every core).
    """
    tiles = np.maximum(np.ceil(counts / 128.0).astype(int), 1)
    t_hi = int(tiles.max())
    t_lo = max(t_hi - 1, 1)
    hi_ids = np.where(tiles == t_hi)[0]
    lo_ids = np.where(tiles < t_hi)[0]
    n_hi = len(hi_ids)
    n_hi_pad = int(64 * np.ceil(n_hi / 64.0)) if n_hi > 0 else 0
    if n_hi_pad > G:
        n_hi_pad = G
    promote = n_hi_pad - n_hi
    if promote > 0:
        hi_ids = np.concatenate([hi_ids, lo_ids[:promote]])
        lo_ids = lo_ids[promote:]
    hi_pc = len(hi_ids) // N_CORES
    lo_pc = len(lo_ids) // N_CORES
    assert hi_pc * N_CORES == len(hi_ids) and lo_pc * N_CORES == len(lo_ids)
    assert hi_pc % SEGS_PER_CHUNK == 0 and lo_pc % SEGS_PER_CHUNK == 0
    perm = np.empty((N_CORES, SPC), dtype=np.int64)
    for c in range(N_CORES):
        perm[c, :lo_pc] = lo_ids[c * lo_pc : (c + 1) * lo_pc]
        perm[c, lo_pc:] = hi_ids[c * hi_pc : (c + 1) * hi_pc]
    chunk_ts = [t_lo] * (lo_pc // SEGS_PER_CHUNK) + [t_hi] * (hi_pc // SEGS_PER_CHUNK)
    return perm, chunk_ts


def _build_graph(chunk_ts):
    import concourse.bass as bass
    import concourse.tile as tile
    from concourse import bacc, mybir

    f32 = mybir.dt.float32
    bf16 = mybir.dt.bfloat16

    n_chunks = len(chunk_ts)
    n_gens = n_chunks // GEN_CHUNKS
    assert n_gens * GEN_CHUNKS == n_chunks
    npad = SEGS_PER_CHUNK * 128 * int(sum(chunk_ts))
    nblk_max = 2 * max(chunk_ts)

    nc = bacc.Bacc(None, target_bir_lowering=False)
    xn_d = nc.dram_tensor("xn", [npad, D], bf16, kind="ExternalInput")
    xt_d = nc.dram_tensor("xt", [D, npad], bf16, kind="ExternalInput")
    ws_d = nc.dram_tensor("wstack", [D, nblk_max, nblk_max], bf16, kind="ExternalInput")
    bv_d = nc.dram_tensor("bvec", [nblk_max, 1], f32, kind="ExternalInput")
    id_d = nc.dram_tensor("ident", [nblk_max, nblk_max], bf16, kind="ExternalInput")
    osum_d = nc.dram_tensor("o_sum", [n_gens, 4, 8 * 512], f32, kind="ExternalOutput")
    omax_d = nc.dram_tensor("o_maxT", [D, SPC], f32, kind="ExternalOutput")

    with tile.TileContext(nc) as tc:
        with (
            tc.tile_pool(name="const", bufs=1) as constp,
            tc.tile_pool(name="xn", bufs=4) as xnp,
            tc.tile_pool(name="xt", bufs=4) as xtp,
            tc.tile_pool(name="wsg", bufs=2) as wsgp,
            tc.tile_pool(name="wcol", bufs=3) as wcolp,
            tc.tile_pool(name="scr", bufs=2) as scrp,
            tc.tile_pool(name="stage", bufs=2) as stagep,
            tc.tile_pool(name="acc", bufs=1) as accp,
            tc.tile_pool(name="psd", bufs=2, space=bass.MemorySpace.PSUM) as psdp,
            tc.tile_pool(name="psw", bufs=2, space=bass.MemorySpace.PSUM) as pswp,
            tc.tile_pool(name="pseg", bufs=1, space=bass.MemorySpace.PSUM) as psegp,
        ):
            ws_sb = constp.tile([D, nblk_max, nblk_max], bf16)
            nc.sync.dma_start(ws_sb[:], ws_d[:])
            bv_sb = constp.tile([nblk_max, 1], f32)
            nc.sync.dma_start(bv_sb[:], bv_d[:])
            id_sb = constp.tile([nblk_max, nblk_max], bf16)
            nc.sync.dma_start(id_sb[:], id_d[:])

            omax_sb = accp.tile([D, SPC], f32)

            chunk_off = np.concatenate(
                [[0], np.cumsum([SEGS_PER_CHUNK * 128 * t for t in chunk_ts])]
            )
            for g in range(n_gens):
                pseg = [
                    psegp.tile([128, 512], f32, tag=f"pseg{b}", name=f"pseg{b}_{g}")
                    for b in range(3)
                ]
                for cc in range(GEN_CHUNKS):
                    c = g * GEN_CHUNKS + cc
                    T = chunk_ts[c]
                    seg_nodes = 128 * T
                    chunk_nodes = SEGS_PER_CHUNK * seg_nodes
                    nblk = chunk_nodes // 512          # = 2T
                    ncombs = SEGS_PER_CHUNK * T
                    row0 = int(chunk_off[c])

                    xn = xnp.tile([128, SEGS_PER_CHUNK, T, D], bf16, tag="xn",
                                  name=f"xn_{c}")
                    nc.sync.dma_start(
                        xn[:],
                        xn_d[row0 : row0 + chunk_nodes, :].rearrange(
                            "(s p q) d -> p s q d", p=128, q=T
                        ),
                    )
                    xt = xtp.tile([D, chunk_nodes], bf16, tag="xt", name=f"xt_{c}")
                    nc.sync.dma_start(xt[:], xt_d[:, row0 : row0 + chunk_nodes])

                    # dots
                    psd = psdp.tile([nblk, 512], f32, tag="psd", name=f"psd_{c}")
                    for r in range(nblk):
                        nc.tensor.matmul(
                            psd[:],
                            ws_sb[:, r, 0:nblk],
                            xt[:, 512 * r : 512 * (r + 1)],
                            start=(r == 0),
                            stop=(r == nblk - 1),
                        )
                    wsg = wsgp.tile([nblk, 512], bf16, tag="wsg", name=f"wsg_{c}")
                    nc.scalar.activation(
                        wsg[:],
                        psd[:],
                        mybir.ActivationFunctionType.Sigmoid,
                        bias=bv_sb[0:nblk, :],
                        scale=1.0,
                    )
                    wcol = wcolp.tile([D, ncombs], bf16, tag="wcol", name=f"wcol_{c}")
                    for q in range(4):
                        psw = pswp.tile([D, nblk], bf16, tag="psw", name=f"psw_{c}_{q}")
                        nc.tensor.transpose(
                            psw[:], wsg[:, 128 * q : 128 * (q + 1)],
                            id_sb[0:nblk, 0:nblk],
                        )
                        nc.scalar.copy(wcol[:, q : ncombs : 4], psw[:])

                    # segment max trees
                    for e in range(SEGS_PER_CHUNK):
                        sl = c * SEGS_PER_CHUNK + e
                        S = xt[:, e * seg_nodes : (e + 1) * seg_nodes]
                        rem = seg_nodes
                        cur = S
                        while rem > 128:
                            half = 128 * ((rem // 128) // 2)
                            if half == 0:
                                break
                            tail = rem - 2 * half
                            nxt = scrp.tile([D, half], bf16, tag=f"scr{half}",
                                            name=f"scr_{sl}_{half}")
                            nc.vector.tensor_tensor(
                                nxt[:], cur[:, 0:half], cur[:, half : 2 * half],
                                op=mybir.AluOpType.max,
                            )
                            if tail > 0:
                                nc.vector.tensor_tensor(
                                    nxt[:, 0:tail], nxt[:, 0:tail],
                                    cur[:, 2 * half : rem],
                                    op=mybir.AluOpType.max,
                                )
                            cur, rem = nxt, half
                        nc.vector.tensor_reduce(
                            omax_sb[:, sl : sl + 1],
                            cur[:, 0:rem],
                            axis=mybir.AxisListType.X,
                            op=mybir.AluOpType.max,
                        )

                    # segment sums (quads)
                    for qd in range(SEGS_PER_CHUNK // 4):
                        e0 = 4 * qd
                        quad = cc * 2 + qd
                        bank, base = quad // 3, 32 * (quad % 3)
                        for k in range(T):
                            m0 = T * e0 + k
                            nc.tensor.matmul(
                                pseg[bank][base : base + 4, :],
                                wcol[:, m0 : m0 + 3 * T + 1 : T],
                                xn[:, e0 : e0 + 4, k, :],
                                start=(k == 0),
                                stop=(k == T - 1),
                            )

                stage = stagep.tile([4, 8 * 512], f32, tag="stage", name=f"stage_{g}")
                for quad in range(8):
                    bank, base = quad // 3, 32 * (quad % 3)
                    nc.scalar.copy(
                        stage[0:4, 512 * quad : 512 * (quad + 1)],
                        pseg[bank][base : base + 4, :],
                    )
                nc.sync.dma_start(osum_d[g, :, :], stage[:])

            nc.sync.dma_start(omax_d[:], omax_sb[:])

    return nc


def _pack_inputs(x, counts, bounds, perm, chunk_ts, attn_w, attn_b):
    bf16 = ml_dtypes.bfloat16
    slot_t = np.repeat(chunk_ts, SEGS_PER_CHUNK)          # T per slot
    slot_rows = 128 * slot_t
    slot_off = np.concatenate([[0], np.cumsum(slot_rows)])
    npad = int(slot_off[-1])
    nblk_max = 2 * max(chunk_ts)

    w = np.asarray(attn_w, dtype=np.float32).reshape(D)
    wstack_np = np.zeros((D, nblk_max, nblk_max), dtype=np.float32)
    for r in range(nblk_max):
        wstack_np[:, r, r] = w
    wstack_np = wstack_np.astype(bf16)
    bvec_np = np.full((nblk_max, 1), float(np.asarray(attn_b).reshape(-1)[0]),
                      np.float32)
    ident_np = np.eye(nblk_max, dtype=np.float32).astype(bf16)

    in_maps = []
    for c in range(N_CORES):
        xpad = np.zeros((npad, D), dtype=np.float32)
        for j in range(SPC):
            s = int(perm[c, j])
            n0, n1 = int(bounds[s]), int(bounds[s + 1])
            if n1 > n0:
                r0 = int(slot_off[j])
                xpad[r0 : r0 + (n1 - n0)] = x[n0:n1]
        xb = xpad.astype(bf16)
        # comb-ordered transpose per slot: col(j, k, p) <- row(off_j + T_j*p + k)
        xtb = np.empty((D, npad), dtype=bf16)
        for j in range(SPC):
            T = int(slot_t[j])
            r0 = int(slot_off[j])
            blk = xb[r0 : r0 + 128 * T].reshape(128, T, D)
            xtb[:, r0 : r0 + 128 * T] = (
                np.transpose(blk, (2, 1, 0)).reshape(D, 128 * T)
            )
        in_maps.append(
            {"xn": xb, "xt": np.ascontiguousarray(xtb), "wstack": wstack_np,
             "bvec": bvec_np, "ident": ident_np}
        )
    return in_maps


LAST_EXEC_NS = None
LAST_RESULT = None


def kernel(x, edge_index, batch, attn_w, attn_b):
    global LAST_EXEC_NS, LAST_RESULT
    from concourse.bass_utils import run_bass_kernel_spmd

    x = np.asarray(x, dtype=np.float32)
    batch = np.asarray(batch)
    bounds = np.searchsorted(batch, np.arange(G + 1))
    counts = np.diff(bounds)
    perm, chunk_ts = _plan(counts)

    nc = _build_graph(chunk_ts)
    nc.compile()
    in_maps = _pack_inputs(x, counts, bounds, perm, chunk_ts, attn_w, attn_b)
    res = run_bass_kernel_spmd(nc, in_maps, core_ids=list(range(N_CORES)))
    LAST_RESULT = res
    LAST_EXEC_NS = getattr(res, "exec_time_ns", None)

    out = np.empty((G, 2 * D), dtype=np.float32)
    for c in range(N_CORES):
        osum = np.asarray(res.results[c]["o_sum"], dtype=np.float32)
        maxT = np.asarray(res.results[c]["o_maxT"], dtype=np.float32)
        for j in range(SPC):
            s = int(perm[c, j])
            g, i2 = divmod(j, 32)
            quad, i = divmod(i2, 4)
            ssum = osum[g, i, 512 * quad + D * i : 512 * quad + D * (i + 1)]
            cnt = float(counts[s])
            out[s, D:] = ssum / max(cnt, 1.0)
            out[s, :D] = maxT[:, j] if cnt > 0 else -np.inf
    return out


# revision 18
# speedup vs baseline: 1.8655x; 1.0895x over previous
"""Trainium2 kernel: AdaptiveFeaturePooling (attention-gated segment mean-pool + max-pool).

v7 strategy (segment-sharded, no collectives):
  - 1024 segments -> 128 per core, but the host REASSIGNS segments to cores so
    every core gets an identical mix: n_hi segments padded to T_HI tiles and
    the rest padded to T_LO tiles (T = ceil(len/128) tiles of 128 nodes).
    One uniform SPMD graph with mixed chunk types; ~6% padding instead of 18%.
  - x ships in bf16 twice:
      "xn": node-major; DMA loads with a T-row interleave so partition p holds
            rows {seg_off + T*p + k} (comb layout, >=2KB DMA runs).
      "xt": d-major, columns permuted into the SAME comb order
            (col = seg_off + 128k + p), ~16KB DMA runs.
    Total HBM bytes ~= one f32 pass over the original input.
  - Per chunk (8 segments):
      dots: 2T PE matmuls, contiguous N=512 rhs from xt, wstack variant r
            stationary, accumulating a [2T x 512] PSUM block; ACT sigmoid;
            4 PE block transposes -> comb weight columns [128 x 8T].
      segsum: 2T PE matmuls; quad of 4 segments per matmul via a [128 x 4]
            stride-T slice of weight columns, rhs xn[:, e0:e0+4, k, :] (N=512);
            accumulates [4 x 512] PSUM regions at 32-aligned bases {0,32,64};
            diagonal 128-blocks are the real sums (host extracts).
      max: VE TT-max tree (bf16 2x mode) per segment over the xt slice.
  - Host: mean = seg_sum / counts, un-permute, assemble [1024, 256] = [max | mean].
"""

import os
import sys

import numpy as np

for _p in ("/opt/trn_rl_repo", "/root/.axon_site/_ro/trn_rl_repo"):
    if os.path.isdir(_p) and _p not in sys.path:
        sys.path.insert(0, _p)

import ml_dtypes  # noqa: E402

N_CORES = 8
G = 1024
SPC = G // N_CORES        # 128 segments per core
D = 128
SEGS_PER_CHUNK = 8
GEN_CHUNKS = 4            # chunks per psum generation (32 segments)


def _plan(counts):
    """Segment -> core assignment with a uniform per-core chunk-type list.

    Returns (perm, chunk_ts) where perm[core, j] = global segment id placed at
    per-core slot j, and chunk_ts = list of T values (tiles/segment), one per
    chunk of 8 slots (same for every core).
    """
    tiles = np.maximum(np.ceil(counts / 128.0).astype(int), 1)
    t_hi = int(tiles.max())
    t_lo = max(t_hi - 1, 1)
    hi_ids = np.where(tiles == t_hi)[0]
    lo_ids = np.where(tiles < t_hi)[0]
    n_hi = len(hi_ids)
    n_hi_pad = int(64 * np.ceil(n_hi / 64.0)) if n_hi > 0 else 0
    if n_hi_pad > G:
        n_hi_pad = G
    promote = n_hi_pad - n_hi
    if promote > 0:
        hi_ids = np.concatenate([hi_ids, lo_ids[:promote]])
        lo_ids = lo_ids[promote:]
    hi_pc = len(hi_ids) // N_CORES
    lo_pc = len(lo_ids) // N_CORES
    assert hi_pc * N_CORES == len(hi_ids) and lo_pc * N_CORES == len(lo_ids)
    assert hi_pc % SEGS_PER_CHUNK == 0 and lo_pc % SEGS_PER_CHUNK == 0
    perm = np.empty((N_CORES, SPC), dtype=np.int64)
    for c in range(N_CORES):
        perm[c, :lo_pc] = lo_ids[c * lo_pc : (c + 1) * lo_pc]
        perm[c, lo_pc:] = hi_ids[c * hi_pc : (c + 1) * hi_pc]
    chunk_ts = [t_lo] * (lo_pc // SEGS_PER_CHUNK) + [t_hi] * (hi_pc // SEGS_PER_CHUNK)
    return perm, chunk_ts


def _build_graph(chunk_ts):
    import concourse.bass as bass
    import concourse.tile as tile
    from concourse import bacc, mybir

    f32 = mybir.dt.float32
    bf16 = mybir.dt.bfloat16

    n_chunks = len(chunk_ts)
    n_gens = n_chunks // GEN_CHUNKS
    assert n_gens * GEN_CHUNKS == n_chunks
    npad = SEGS_PER_CHUNK * 128 * int(sum(chunk_ts))
    nblk_max = 2 * max(chunk_ts)

    nc = bacc.Bacc(None, target_bir_lowering=False)
    xn_d = nc.dram_tensor("xn", [npad, D], bf16, kind="ExternalInput")
    xt_d = nc.dram_tensor("xt", [D, npad], bf16, kind="ExternalInput")
    ws_d = nc.dram_tensor("wstack", [D, nblk_max, nblk_max], bf16, kind="ExternalInput")
    bv_d = nc.dram_tensor("bvec", [nblk_max, 1], f32, kind="ExternalInput")
    id_d = nc.dram_tensor("ident", [nblk_max, nblk_max], bf16, kind="ExternalInput")
    osum_d = nc.dram_tensor("o_sum", [n_gens, 4, 8 * 512], f32, kind="ExternalOutput")
    omax_d = nc.dram_tensor("o_maxT", [D, SPC], f32, kind="ExternalOutput")

    with tile.TileContext(nc) as tc:
        with (
            tc.tile_pool(name="const", bufs=1) as constp,
            tc.tile_pool(name="xn", bufs=4) as xnp,
            tc.tile_pool(name="xt", bufs=4) as xtp,
            tc.tile_pool(name="wsg", bufs=2) as wsgp,
            tc.tile_pool(name="wcol", bufs=3) as wcolp,
            tc.tile_pool(name="scr", bufs=2) as scrp,
            tc.tile_pool(name="stage", bufs=2) as stagep,
            tc.tile_pool(name="acc", bufs=1) as accp,
            tc.tile_pool(name="psd", bufs=1, space=bass.MemorySpace.PSUM) as psdp,
            tc.tile_pool(name="psw", bufs=1, space=bass.MemorySpace.PSUM) as pswp,
            tc.tile_pool(name="pseg", bufs=2, space=bass.MemorySpace.PSUM) as psegp,
        ):
            ws_sb = constp.tile([D, nblk_max, nblk_max], bf16)
            nc.sync.dma_start(ws_sb[:], ws_d[:])
            bv_sb = constp.tile([nblk_max, 1], f32)
            nc.sync.dma_start(bv_sb[:], bv_d[:])
            id_sb = constp.tile([nblk_max, nblk_max], bf16)
            nc.sync.dma_start(id_sb[:], id_d[:])

            omax_sb = accp.tile([D, SPC], f32)

            chunk_off = np.concatenate(
                [[0], np.cumsum([SEGS_PER_CHUNK * 128 * t for t in chunk_ts])]
            )
            for g in range(n_gens):
                pseg = [
                    psegp.tile([128, 512], f32, tag=f"pseg{b}", name=f"pseg{b}_{g}")
                    for b in range(3)
                ]
                stage = stagep.tile([4, 8 * 512], f32, tag="stage", name=f"stage_{g}")
                for cc in range(GEN_CHUNKS):
                    c = g * GEN_CHUNKS + cc
                    T = chunk_ts[c]
                    seg_nodes = 128 * T
                    chunk_nodes = SEGS_PER_CHUNK * seg_nodes
                    nblk = chunk_nodes // 512          # = 2T
                    ncombs = SEGS_PER_CHUNK * T
                    row0 = int(chunk_off[c])

                    xn = xnp.tile([128, SEGS_PER_CHUNK, T, D], bf16, tag="xn",
                                  name=f"xn_{c}")
                    nc.sync.dma_start(
                        xn[:],
                        xn_d[row0 : row0 + chunk_nodes, :].rearrange(
                            "(s p q) d -> p s q d", p=128, q=T
                        ),
                    )
                    xt = xtp.tile([D, chunk_nodes], bf16, tag="xt", name=f"xt_{c}")
                    nc.sync.dma_start(xt[:], xt_d[:, row0 : row0 + chunk_nodes])

                    # dots
                    psd = psdp.tile([nblk, 512], f32, tag="psd", name=f"psd_{c}")
                    for r in range(nblk):
                        nc.tensor.matmul(
                            psd[:],
                            ws_sb[:, r, 0:nblk],
                            xt[:, 512 * r : 512 * (r + 1)],
                            start=(r == 0),
                            stop=(r == nblk - 1),
                        )
                    wsg = wsgp.tile([nblk, 512], bf16, tag="wsg", name=f"wsg_{c}")
                    nc.scalar.activation(
                        wsg[:],
                        psd[:],
                        mybir.ActivationFunctionType.Sigmoid,
                        bias=bv_sb[0:nblk, :],
                        scale=1.0,
                    )
                    wcol = wcolp.tile([D, ncombs], bf16, tag="wcol", name=f"wcol_{c}")
                    for q in range(4):
                        psw = pswp.tile([D, nblk], bf16, tag="psw", name=f"psw_{c}_{q}")
                        nc.tensor.transpose(
                            psw[:], wsg[:, 128 * q : 128 * (q + 1)],
                            id_sb[0:nblk, 0:nblk],
                        )
                        nc.scalar.copy(wcol[:, q : ncombs : 4], psw[:])

                    # segment max trees
                    for e in range(SEGS_PER_CHUNK):
                        sl = c * SEGS_PER_CHUNK + e
                        S = xt[:, e * seg_nodes : (e + 1) * seg_nodes]
                        rem = seg_nodes
                        cur = S
                        while rem > 128:
                            half = 128 * ((rem // 128) // 2)
                            if half == 0:
                                break
                            tail = rem - 2 * half
                            nxt = scrp.tile([D, half], bf16, tag=f"scr{half}",
                                            name=f"scr_{sl}_{half}")
                            nc.vector.tensor_tensor(
                                nxt[:], cur[:, 0:half], cur[:, half : 2 * half],
                                op=mybir.AluOpType.max,
                            )
                            if tail > 0:
                                nc.vector.tensor_tensor(
                                    nxt[:, 0:tail], nxt[:, 0:tail],
                                    cur[:, 2 * half : rem],
                                    op=mybir.AluOpType.max,
                                )
                            cur, rem = nxt, half
                        nc.vector.tensor_reduce(
                            omax_sb[:, sl : sl + 1],
                            cur[:, 0:rem],
                            axis=mybir.AxisListType.X,
                            op=mybir.AluOpType.max,
                        )

                    # segment sums (quads)
                    for qd in range(SEGS_PER_CHUNK // 4):
                        e0 = 4 * qd
                        quad = cc * 2 + qd
                        bank, base = quad // 3, 32 * (quad % 3)
                        for k in range(T):
                            m0 = T * e0 + k
                            nc.tensor.matmul(
                                pseg[bank][base : base + 4, :],
                                wcol[:, m0 : m0 + 3 * T + 1 : T],
                                xn[:, e0 : e0 + 4, k, :],
                                start=(k == 0),
                                stop=(k == T - 1),
                            )

                for quad in range(8):
                    bank, base = quad // 3, 32 * (quad % 3)
                    nc.scalar.copy(
                        stage[0:4, 512 * quad : 512 * (quad + 1)],
                        pseg[bank][base : base + 4, :],
                    )
                nc.sync.dma_start(osum_d[g, :, :], stage[:])

            nc.sync.dma_start(omax_d[:], omax_sb[:])

    return nc


def _pack_inputs(x, counts, bounds, perm, chunk_ts, attn_w, attn_b):
    bf16 = ml_dtypes.bfloat16
    slot_t = np.repeat(chunk_ts, SEGS_PER_CHUNK)          # T per slot
    slot_rows = 128 * slot_t
    slot_off = np.concatenate([[0], np.cumsum(slot_rows)])
    npad = int(slot_off[-1])
    nblk_max = 2 * max(chunk_ts)

    w = np.asarray(attn_w, dtype=np.float32).reshape(D)
    wstack_np = np.zeros((D, nblk_max, nblk_max), dtype=np.float32)
    for r in range(nblk_max):
        wstack_np[:, r, r] = w
    wstack_np = wstack_np.astype(bf16)
    bvec_np = np.full((nblk_max, 1), float(np.asarray(attn_b).reshape(-1)[0]),
                      np.float32)
    ident_np = np.eye(nblk_max, dtype=np.float32).astype(bf16)

    in_maps = []
    for c in range(N_CORES):
        xpad = np.zeros((npad, D), dtype=np.float32)
        for j in range(SPC):
            s = int(perm[c, j])
            n0, n1 = int(bounds[s]), int(bounds[s + 1])
            if n1 > n0:
                r0 = int(slot_off[j])
                xpad[r0 : r0 + (n1 - n0)] = x[n0:n1]
        xb = xpad.astype(bf16)
        # comb-ordered transpose per slot: col(j, k, p) <- row(off_j + T_j*p + k)
        xtb = np.empty((D, npad), dtype=bf16)
        for j in range(SPC):
            T = int(slot_t[j])
            r0 = int(slot_off[j])
            blk = xb[r0 : r0 + 128 * T].reshape(128, T, D)
            xtb[:, r0 : r0 + 128 * T] = (
                np.transpose(blk, (2, 1, 0)).reshape(D, 128 * T)
            )
        in_maps.append(
            {"xn": xb, "xt": np.ascontiguousarray(xtb), "wstack": wstack_np,
             "bvec": bvec_np, "ident": ident_np}
        )
    return in_maps


LAST_EXEC_NS = None
LAST_RESULT = None


def kernel(x, edge_index, batch, attn_w, attn_b):
    global LAST_EXEC_NS, LAST_RESULT
    from concourse.bass_utils import run_bass_kernel_spmd

    x = np.asarray(x, dtype=np.float32)
    batch = np.asarray(batch)
    bounds = np.searchsorted(batch, np.arange(G + 1))
    counts = np.diff(bounds)
    perm, chunk_ts = _plan(counts)

    nc = _build_graph(chunk_ts)
    nc.compile()
    in_maps = _pack_inputs(x, counts, bounds, perm, chunk_ts, attn_w, attn_b)
    res = run_bass_kernel_spmd(nc, in_maps, core_ids=list(range(N_CORES)))
    LAST_RESULT = res
    LAST_EXEC_NS = getattr(res, "exec_time_ns", None)

    out = np.empty((G, 2 * D), dtype=np.float32)
    for c in range(N_CORES):
        osum = np.asarray(res.results[c]["o_sum"], dtype=np.float32)
        maxT = np.asarray(res.results[c]["o_maxT"], dtype=np.float32)
        for j in range(SPC):
            s = int(perm[c, j])
            g, i2 = divmod(j, 32)
            quad, i = divmod(i2, 4)
            ssum = osum[g, i, 512 * quad + D * i : 512 * quad + D * (i + 1)]
            cnt = float(counts[s])
            out[s, D:] = ssum / max(cnt, 1.0)
            out[s, :D] = maxT[:, j] if cnt > 0 else -np.inf
    return out


# revision 19
# speedup vs baseline: 1.9281x; 1.0336x over previous
"""Trainium2 kernel: AdaptiveFeaturePooling (attention-gated segment mean-pool + max-pool).

v7 strategy (segment-sharded, no collectives):
  - 1024 segments -> 128 per core, but the host REASSIGNS segments to cores so
    every core gets an identical mix: n_hi segments padded to T_HI tiles and
    the rest padded to T_LO tiles (T = ceil(len/128) tiles of 128 nodes).
    One uniform SPMD graph with mixed chunk types; ~6% padding instead of 18%.
  - x ships in bf16 twice:
      "xn": node-major; DMA loads with a T-row interleave so partition p holds
            rows {seg_off + T*p + k} (comb layout, >=2KB DMA runs).
      "xt": d-major, columns permuted into the SAME comb order
            (col = seg_off + 128k + p), ~16KB DMA runs.
    Total HBM bytes ~= one f32 pass over the original input.
  - Per chunk (8 segments):
      dots: 2T PE matmuls, contiguous N=512 rhs from xt, wstack variant r
            stationary, accumulating a [2T x 512] PSUM block; ACT sigmoid;
            4 PE block transposes -> comb weight columns [128 x 8T].
      segsum: 2T PE matmuls; quad of 4 segments per matmul via a [128 x 4]
            stride-T slice of weight columns, rhs xn[:, e0:e0+4, k, :] (N=512);
            accumulates [4 x 512] PSUM regions at 32-aligned bases {0,32,64};
            diagonal 128-blocks are the real sums (host extracts).
      max: VE TT-max tree (bf16 2x mode) per segment over the xt slice.
  - Host: mean = seg_sum / counts, un-permute, assemble [1024, 256] = [max | mean].
"""

import os
import sys

import numpy as np

for _p in ("/opt/trn_rl_repo", "/root/.axon_site/_ro/trn_rl_repo"):
    if os.path.isdir(_p) and _p not in sys.path:
        sys.path.insert(0, _p)

import ml_dtypes  # noqa: E402

N_CORES = 8
G = 1024
SPC = G // N_CORES        # 128 segments per core
D = 128
SEGS_PER_CHUNK = 8
GEN_CHUNKS = 4            # chunks per psum generation (32 segments)


def _plan(counts):
    """Segment -> core assignment with a uniform per-core chunk-type list.

    Returns (perm, chunk_ts) where perm[core, j] = global segment id placed at
    per-core slot j, and chunk_ts = list of T values (tiles/segment), one per
    chunk of 8 slots (same for every core).
    """
    tiles = np.maximum(np.ceil(counts / 128.0).astype(int), 1)
    t_hi = int(tiles.max())
    t_lo = max(t_hi - 1, 1)
    hi_ids = np.where(tiles == t_hi)[0]
    lo_ids = np.where(tiles < t_hi)[0]
    n_hi = len(hi_ids)
    n_hi_pad = int(64 * np.ceil(n_hi / 64.0)) if n_hi > 0 else 0
    if n_hi_pad > G:
        n_hi_pad = G
    promote = n_hi_pad - n_hi
    if promote > 0:
        hi_ids = np.concatenate([hi_ids, lo_ids[:promote]])
        lo_ids = lo_ids[promote:]
    hi_pc = len(hi_ids) // N_CORES
    lo_pc = len(lo_ids) // N_CORES
    assert hi_pc * N_CORES == len(hi_ids) and lo_pc * N_CORES == len(lo_ids)
    assert hi_pc % SEGS_PER_CHUNK == 0 and lo_pc % SEGS_PER_CHUNK == 0
    perm = np.empty((N_CORES, SPC), dtype=np.int64)
    for c in range(N_CORES):
        perm[c, :lo_pc] = lo_ids[c * lo_pc : (c + 1) * lo_pc]
        perm[c, lo_pc:] = hi_ids[c * hi_pc : (c + 1) * hi_pc]
    chunk_ts = [t_lo] * (lo_pc // SEGS_PER_CHUNK) + [t_hi] * (hi_pc // SEGS_PER_CHUNK)
    return perm, chunk_ts


def _build_graph(chunk_ts):
    import concourse.bass as bass
    import concourse.tile as tile
    from concourse import bacc, mybir

    f32 = mybir.dt.float32
    bf16 = mybir.dt.bfloat16

    n_chunks = len(chunk_ts)
    n_gens = n_chunks // GEN_CHUNKS
    assert n_gens * GEN_CHUNKS == n_chunks
    npad = SEGS_PER_CHUNK * 128 * int(sum(chunk_ts))
    nblk_max = 2 * max(chunk_ts)

    nc = bacc.Bacc(None, target_bir_lowering=False)
    xn_d = nc.dram_tensor("xn", [npad, D], bf16, kind="ExternalInput")
    xt_d = nc.dram_tensor("xt", [D, npad], bf16, kind="ExternalInput")
    ws_d = nc.dram_tensor("wstack", [D, nblk_max, nblk_max], bf16, kind="ExternalInput")
    bv_d = nc.dram_tensor("bvec", [nblk_max, 1], f32, kind="ExternalInput")
    id_d = nc.dram_tensor("ident", [nblk_max, nblk_max], bf16, kind="ExternalInput")
    osum_d = nc.dram_tensor("o_sum", [n_gens, 4, 8 * 512], f32, kind="ExternalOutput")
    omax_d = nc.dram_tensor("o_maxT", [D, SPC], f32, kind="ExternalOutput")

    with tile.TileContext(nc) as tc:
        with (
            tc.tile_pool(name="const", bufs=1) as constp,
            tc.tile_pool(name="xn", bufs=5) as xnp,
            tc.tile_pool(name="xt", bufs=5) as xtp,
            tc.tile_pool(name="wsg", bufs=2) as wsgp,
            tc.tile_pool(name="wcol", bufs=3) as wcolp,
            tc.tile_pool(name="scr", bufs=2) as scrp,
            tc.tile_pool(name="stage", bufs=1) as stagep,
            tc.tile_pool(name="acc", bufs=1) as accp,
            tc.tile_pool(name="psd", bufs=1, space=bass.MemorySpace.PSUM) as psdp,
            tc.tile_pool(name="psw", bufs=1, space=bass.MemorySpace.PSUM) as pswp,
            tc.tile_pool(name="pseg", bufs=2, space=bass.MemorySpace.PSUM) as psegp,
        ):
            ws_sb = constp.tile([D, nblk_max, nblk_max], bf16)
            nc.gpsimd.dma_start(ws_sb[:], ws_d[:])
            bv_sb = constp.tile([nblk_max, 1], f32)
            nc.gpsimd.dma_start(bv_sb[:], bv_d[:])
            id_sb = constp.tile([nblk_max, nblk_max], bf16)
            nc.gpsimd.dma_start(id_sb[:], id_d[:])

            omax_sb = accp.tile([D, SPC], f32)

            chunk_off = np.concatenate(
                [[0], np.cumsum([SEGS_PER_CHUNK * 128 * t for t in chunk_ts])]
            )
            for g in range(n_gens):
                pseg = [
                    psegp.tile([128, 512], f32, tag=f"pseg{b}", name=f"pseg{b}_{g}")
                    for b in range(3)
                ]
                stage = stagep.tile([4, 8 * 512], f32, tag="stage", name=f"stage_{g}")
                for cc in range(GEN_CHUNKS):
                    c = g * GEN_CHUNKS + cc
                    T = chunk_ts[c]
                    seg_nodes = 128 * T
                    chunk_nodes = SEGS_PER_CHUNK * seg_nodes
                    nblk = chunk_nodes // 512          # = 2T
                    ncombs = SEGS_PER_CHUNK * T
                    row0 = int(chunk_off[c])

                    xn = xnp.tile([128, SEGS_PER_CHUNK, T, D], bf16, tag="xn",
                                  name=f"xn_{c}")
                    nc.sync.dma_start(
                        xn[:],
                        xn_d[row0 : row0 + chunk_nodes, :].rearrange(
                            "(s p q) d -> p s q d", p=128, q=T
                        ),
                    )
                    xt = xtp.tile([D, chunk_nodes], bf16, tag="xt", name=f"xt_{c}")
                    nc.sync.dma_start(xt[:], xt_d[:, row0 : row0 + chunk_nodes])

                    # dots
                    psd = psdp.tile([nblk, 512], f32, tag="psd", name=f"psd_{c}")
                    for r in range(nblk):
                        nc.tensor.matmul(
                            psd[:],
                            ws_sb[:, r, 0:nblk],
                            xt[:, 512 * r : 512 * (r + 1)],
                            start=(r == 0),
                            stop=(r == nblk - 1),
                        )
                    wsg = wsgp.tile([nblk, 512], bf16, tag="wsg", name=f"wsg_{c}")
                    nc.scalar.activation(
                        wsg[:],
                        psd[:],
                        mybir.ActivationFunctionType.Sigmoid,
                        bias=bv_sb[0:nblk, :],
                        scale=1.0,
                    )
                    wcol = wcolp.tile([D, ncombs], bf16, tag="wcol", name=f"wcol_{c}")
                    for q in range(4):
                        psw = pswp.tile([D, nblk], bf16, tag="psw", name=f"psw_{c}_{q}")
                        nc.tensor.transpose(
                            psw[:], wsg[:, 128 * q : 128 * (q + 1)],
                            id_sb[0:nblk, 0:nblk],
                        )
                        nc.scalar.copy(wcol[:, q : ncombs : 4], psw[:])

                    # segment max trees
                    for e in range(SEGS_PER_CHUNK):
                        sl = c * SEGS_PER_CHUNK + e
                        S = xt[:, e * seg_nodes : (e + 1) * seg_nodes]
                        rem = seg_nodes
                        cur = S
                        while rem > 128:
                            half = 128 * ((rem // 128) // 2)
                            if half == 0:
                                break
                            tail = rem - 2 * half
                            nxt = scrp.tile([D, half], bf16, tag=f"scr{half}",
                                            name=f"scr_{sl}_{half}")
                            nc.vector.tensor_tensor(
                                nxt[:], cur[:, 0:half], cur[:, half : 2 * half],
                                op=mybir.AluOpType.max,
                            )
                            if tail > 0:
                                nc.vector.tensor_tensor(
                                    nxt[:, 0:tail], nxt[:, 0:tail],
                                    cur[:, 2 * half : rem],
                                    op=mybir.AluOpType.max,
                                )
                            cur, rem = nxt, half
                        nc.vector.tensor_reduce(
                            omax_sb[:, sl : sl + 1],
                            cur[:, 0:rem],
                            axis=mybir.AxisListType.X,
                            op=mybir.AluOpType.max,
                        )

                    # segment sums (quads)
                    for qd in range(SEGS_PER_CHUNK // 4):
                        e0 = 4 * qd
                        quad = cc * 2 + qd
                        bank, base = quad // 3, 32 * (quad % 3)
                        for k in range(T):
                            m0 = T * e0 + k
                            nc.tensor.matmul(
                                pseg[bank][base : base + 4, :],
                                wcol[:, m0 : m0 + 3 * T + 1 : T],
                                xn[:, e0 : e0 + 4, k, :],
                                start=(k == 0),
                                stop=(k == T - 1),
                            )

                for quad in range(8):
                    bank, base = quad // 3, 32 * (quad % 3)
                    nc.scalar.copy(
                        stage[0:4, 512 * quad : 512 * (quad + 1)],
                        pseg[bank][base : base + 4, :],
                    )
                nc.sync.dma_start(osum_d[g, :, :], stage[:])

            nc.sync.dma_start(omax_d[:], omax_sb[:])

    return nc


def _pack_inputs(x, counts, bounds, perm, chunk_ts, attn_w, attn_b):
    bf16 = ml_dtypes.bfloat16
    slot_t = np.repeat(chunk_ts, SEGS_PER_CHUNK)          # T per slot
    slot_rows = 128 * slot_t
    slot_off = np.concatenate([[0], np.cumsum(slot_rows)])
    npad = int(slot_off[-1])
    nblk_max = 2 * max(chunk_ts)

    w = np.asarray(attn_w, dtype=np.float32).reshape(D)
    wstack_np = np.zeros((D, nblk_max, nblk_max), dtype=np.float32)
    for r in range(nblk_max):
        wstack_np[:, r, r] = w
    wstack_np = wstack_np.astype(bf16)
    bvec_np = np.full((nblk_max, 1), float(np.asarray(attn_b).reshape(-1)[0]),
                      np.float32)
    ident_np = np.eye(nblk_max, dtype=np.float32).astype(bf16)

    in_maps = []
    for c in range(N_CORES):
        xpad = np.zeros((npad, D), dtype=np.float32)
        for j in range(SPC):
            s = int(perm[c, j])
            n0, n1 = int(bounds[s]), int(bounds[s + 1])
            if n1 > n0:
                r0 = int(slot_off[j])
                xpad[r0 : r0 + (n1 - n0)] = x[n0:n1]
        xb = xpad.astype(bf16)
        # comb-ordered transpose per slot: col(j, k, p) <- row(off_j + T_j*p + k)
        xtb = np.empty((D, npad), dtype=bf16)
        for j in range(SPC):
            T = int(slot_t[j])
            r0 = int(slot_off[j])
            blk = xb[r0 : r0 + 128 * T].reshape(128, T, D)
            xtb[:, r0 : r0 + 128 * T] = (
                np.transpose(blk, (2, 1, 0)).reshape(D, 128 * T)
            )
        in_maps.append(
            {"xn": xb, "xt": np.ascontiguousarray(xtb), "wstack": wstack_np,
             "bvec": bvec_np, "ident": ident_np}
        )
    return in_maps


LAST_EXEC_NS = None
LAST_RESULT = None


def kernel(x, edge_index, batch, attn_w, attn_b):
    global LAST_EXEC_NS, LAST_RESULT
    from concourse.bass_utils import run_bass_kernel_spmd

    x = np.asarray(x, dtype=np.float32)
    batch = np.asarray(batch)
    bounds = np.searchsorted(batch, np.arange(G + 1))
    counts = np.diff(bounds)
    perm, chunk_ts = _plan(counts)

    nc = _build_graph(chunk_ts)
    nc.compile()
    in_maps = _pack_inputs(x, counts, bounds, perm, chunk_ts, attn_w, attn_b)
    res = run_bass_kernel_spmd(nc, in_maps, core_ids=list(range(N_CORES)))
    LAST_RESULT = res
    LAST_EXEC_NS = getattr(res, "exec_time_ns", None)

    out = np.empty((G, 2 * D), dtype=np.float32)
    for c in range(N_CORES):
        osum = np.asarray(res.results[c]["o_sum"], dtype=np.float32)
        maxT = np.asarray(res.results[c]["o_maxT"], dtype=np.float32)
        for j in range(SPC):
            s = int(perm[c, j])
            g, i2 = divmod(j, 32)
            quad, i = divmod(i2, 4)
            ssum = osum[g, i, 512 * quad + D * i : 512 * quad + D * (i + 1)]
            cnt = float(counts[s])
            out[s, D:] = ssum / max(cnt, 1.0)
            out[s, :D] = maxT[:, j] if cnt > 0 else -np.inf
    return out
